# revision 64
# baseline (speedup 1.0000x reference)
"""Trainium2 Bass kernel for nn_Boundary_Enchance (dense_cnn).

Pure data parallel: core i of 8 processes batch image i.  The heavy matmul
work runs in fp8(e4m3) DoubleRow mode (2 weight planes per PE cell, K_eff=256,
0.5 cycles/col), 4x the bf16 column rate:

  - fuse 1x1 conv (5->16 + bias + relu): one DoubleRow matmul per 8-row
    strip; pair halves = y rows 0..3 / rows 4..7 (+ ones partition for the
    bias).  Evacuated (relu, fp8) by the Vector engine with accum_out row
    sums feeding the global-average-pool / SE path.
  - 3x3 conv over concat(x, fuse): 3 DoubleRow matmuls per strip (one per
    dx tap); pair halves = (x rows, fuse rows) interleaved per strip in one
    big SBUF region so the pair stride is a constant 512.  dy taps are
    packed row-Toeplitz in the 128 partitions; SAME padding via
    partial-column accumulating matmuls.  Evacuated (bias+relu, fp8) by the
    Scalar engine, 2 strips per op, into the fcc region.
  - mask head: one plain fp8 matmul per 2 strips (K=97: 96 fcc rows +
    ones; M=6 mask-diff logits; N=1024) — SE-independent, so it interleaves
    with the conv pipeline.  Three matmuls share a 2-bank PSUM tile at
    partition bases 0/32/64; one Copy op + one DMA evacuates 6 strips of
    logits as bf16.
  - SE chain stays on device (row sums -> selection matmul -> MLP ->
    sigmoid); the 128-wide sigmoid vector is DMA'd back.

The host does layout packing (fp8 Toeplitz tiles) and the cheap epilogue:
sigmoid on the mask logits, the 5-channel boundary head (1x1 conv with
device-provided SE scale + sigmoid), add, clip, and the final rank-1 1x1
16-channel expansion (out = cv_w * s + cv_b).
"""

import numpy as np
import ml_dtypes

F8 = ml_dtypes.float8_e4m3
BF16 = ml_dtypes.bfloat16

H = 512
W = 512
SB = 6                     # output rows per strip
NT = (H + SB - 1) // SB    # 86 strips
NV = NT // 2               # 43 mask matmuls (2 strips each)
NPIX = float(H * W)
MGS = 6                    # strips per logit tile (3 or 6)
CONV_BUFS = 2
COPY_PRIO = 0              # priority offset for logit copies (0 = none)
TBL_PRELOAD = True         # dummy activation to preload the act table early
FUSE_PRIO = True           # schedule fuse matmuls ahead of all other PE work
MASK_BUFS = 1
COPY_SPLIT = True
COPY_MOD = 4
OUT_SP = True              # out DMAs on the SP queue instead of Act
DVE_TAIL = 4               # last conv-evac pairs handled by DVE
SGB = 5                    # logit staging ring depth
FACT_MOD = 0               # every Nth fuse evac on Act (0 = all on DVE)
NG = (NT + MGS - 1) // MGS
NYR = 32                   # y ring depth (strips)
FL = 3                     # fuse -> conv front lag (strips)
ML = 2                     # conv-evac pair -> mask matmul lag (pairs)

_cache = {}


# ----------------------------------------------------------------------------
# host-side weight layout builders
# ----------------------------------------------------------------------------

def _conv_pair_lhsT(fc_w):
    """[3][128, 192]: cols 0-95 x-half, 96-191 F-half.
    W[dx][r*16+c, half*96 + i*16+oc] = fc_w[oc, half*16+c, r-i, dx]."""
    out = np.zeros((3, 128, 192), np.float32)
    for dx in range(3):
        for half in range(2):
            for i in range(SB):
                for ky in range(3):
                    r = i + ky
                    out[dx, r * 16:r * 16 + 16,
                        half * 96 + i * 16:half * 96 + i * 16 + 16] = \
                        fc_w[:, half * 16:half * 16 + 16, ky, dx].T
    return out


def _fuse_pair_lhsT(fuse_w, fuse_b, zero_out_rows=()):
    """[21, 256]: half0 (cols 0-127) = y rows 0-3 + bias on partition 20;
    half1 (cols 128-255) = y rows 4-7.  lhsT[r%4*5+yc, half*128 + r*16+oc].
    zero_out_rows: tile rows r whose output (and bias) must be zeroed."""
    out = np.zeros((21, 256), np.float32)
    for r in range(8):
        if r in zero_out_rows:
            continue
        half = r // 4
        q = (r % 4) * 5
        out[q:q + 5, half * 128 + r * 16:half * 128 + r * 16 + 16] = \
            fuse_w[:, :, 0, 0].T
        out[20, r * 16:r * 16 + 16] = fuse_b  # bias lives in half0
    return out


def _lm6(fm_w, fm_b):
    """Static mask head [128, 6]: cols = the 6 output rows of a strip."""
    lm = np.zeros((128, 6), np.float32)
    dm = fm_w[1, :, 0, 0] - fm_w[0, :, 0, 0]
    for i in range(SB):
        lm[i * 16:i * 16 + 16, i] = dm
    lm[96, 0:6] = fm_b[1] - fm_b[0]
    return lm


def _se_consts(se_w1, se_w2):
    """[128, 160] f32: SEL (cols 0-15), W1L (16-31), W2R (32-159)."""
    out = np.zeros((128, 160), np.float32)
    for r in range(1, 7):
        for fc in range(16):
            out[r * 16 + fc, fc] = 1.0 / NPIX
    out[0:16, 16:32] = se_w1.T
    out[0:16, 32 + 97:32 + 102] = se_w2.T
    return out


def _fcb_col(fc_b):
    out = np.zeros((96, 1), np.float32)
    for i in range(SB):
        out[i * 16:(i + 1) * 16, 0] = fc_b
    return out


def _pack_w8(fuse_w, fuse_b, fc_w, fm_w, fm_b):
    """[128, 1360] fp8: 3 conv pair blocks (192 each), 3 fuse variants
    (256 each): LF, LF_first (row -1 zeroed), LF_last (rows 3.. zeroed),
    then the static mask head LM6 (cols 1344-1349)."""
    out = np.zeros((128, 1360), np.float32)
    cw = _conv_pair_lhsT(fc_w)
    for dx in range(3):
        out[:, dx * 192:(dx + 1) * 192] = cw[dx]
    base = 3 * 192
    out[0:21, base:base + 256] = _fuse_pair_lhsT(fuse_w, fuse_b)
    out[0:21, base + 256:base + 512] = _fuse_pair_lhsT(
        fuse_w, fuse_b, zero_out_rows=(0,))
    out[0:21, base + 512:base + 768] = _fuse_pair_lhsT(
        fuse_w, fuse_b, zero_out_rows=(3, 4, 5, 6, 7))
    out[:, 1344:1350] = _lm6(fm_w, fm_b)
    return out.astype(F8)


# ----------------------------------------------------------------------------
# bass graph
# ----------------------------------------------------------------------------

def _build():
    import concourse.bass as bass
    import concourse.bacc as bacc
    import concourse.tile as tile
    from concourse import mybir

    f32 = mybir.dt.float32
    bf16 = mybir.dt.bfloat16
    fp8 = mybir.dt.float8e4
    AF = mybir.ActivationFunctionType
    ALU = mybir.AluOpType
    DR = mybir.MatmulPerfMode.DoubleRow

    nc = bacc.Bacc("TRN2", target_bir_lowering=False)
    xp_ext = nc.declare_dram_parameter("xp", [128, NT * W], fp8, isOutput=False)
    yp_ext = nc.declare_dram_parameter("yp", [21, NT * 2 * W], fp8,
                                       isOutput=False)
    w8_ext = nc.declare_dram_parameter("w8", [128, 1360], fp8, isOutput=False)
    w32_ext = nc.declare_dram_parameter("w32", [128, 160], f32, isOutput=False)
    fcb_ext = nc.declare_dram_parameter("fcb", [96, 1], f32, isOutput=False)
    ct8_ext = nc.declare_dram_parameter("ct8", [1, NT * W], fp8, isOutput=False)
    out_ext = nc.declare_dram_parameter("outp", [NG, 70, (MGS // 3) * W],
                                        bf16, isOutput=True)
    sep_ext = nc.declare_dram_parameter("sep", [128, 1], f32, isOutput=True)

    with tile.TileContext(nc) as tc:
        with (
            tc.tile_pool(name="singles", bufs=1) as singles,
            tc.tile_pool(name="sgring", bufs=SGB) as sgring,
            tc.tile_pool(name="ps_fuse", bufs=2, space="PSUM") as ps_fuse,
            tc.tile_pool(name="ps_conv", bufs=CONV_BUFS,
                         space="PSUM") as ps_conv,
            tc.tile_pool(name="ps_mask", bufs=MASK_BUFS,
                         space="PSUM") as ps_mask,
        ):
            # ---------------- constants + first data chunks -------------
            w8 = singles.tile([128, 1360], fp8, tag="w8")
            nc.sync.dma_start(out=w8[:, :], in_=w8_ext[:, :])
            if TBL_PRELOAD:
                # touch Relu early so the act-table load is off the
                # critical path of the first conv evacuation
                tpre = singles.tile([1, 1], f32, tag="tpre")
                nc.scalar.activation(out=tpre[:, :], in_=tpre[:, :],
                                     func=AF.Relu)
            yR = singles.tile([21, NYR * 2 * W], fp8, tag="yR")
            xf = singles.tile([128, NT * 2 * W], fp8, tag="xf")
            fcc = singles.tile([128, NT * W], fp8, tag="fcc")

            def x_range(s0, n):
                v = xf[:, 2 * s0 * W:2 * (s0 + n) * W] \
                    .rearrange("p (s j) -> p s j", j=2 * W)[:, :, 0:W]
                nc.sync.dma_start(
                    out=v,
                    in_=xp_ext[:, s0 * W:(s0 + n) * W]
                    .rearrange("p (s j) -> p s j", j=W))

            def x_chunk(k):
                x_range(16 * k, min(16, NT - 16 * k))

            def y_chunk(k):
                n = min(8, NT - 8 * k)
                r0 = (8 * k) % NYR
                nc.sync.dma_start(
                    out=yR[:, r0 * 2 * W:(r0 + n) * 2 * W],
                    in_=yp_ext[:, 8 * k * 2 * W:(8 * k + n) * 2 * W])

            y_chunk(0)
            y_chunk(1)
            x_range(0, 8)
            fcb = singles.tile([96, 1], f32, tag="fcb")
            nc.sync.dma_start(out=fcb[:, :], in_=fcb_ext[:, :])
            w32 = singles.tile([128, 160], f32, tag="w32")
            nc.sync.dma_start(out=w32[:, :], in_=w32_ext[:, :])
            y_chunk(2)
            x_range(8, 8)
            nc.sync.dma_start(out=fcc[96:97, :], in_=ct8_ext[:, :])
            y_chunk(3)

            WDR = [w8[:, dx * 192:(dx + 1) * 192]
                   .rearrange("p (two m) -> p two m", two=2) for dx in range(3)]
            fb = 3 * 192
            LFV = [w8[0:21, fb + v * 256:fb + (v + 1) * 256]
                   .rearrange("p (two m) -> p two m", two=2) for v in range(3)]
            LM6 = w8[0:97, 1344:1350]
            SEL = w32[:, 0:16]
            W1L = w32[0:16, 16:32]
            W2R = w32[0:16, 32:160]

            Ra = singles.tile([128, NT], f32, tag="Ra")

            # ---------------- pipeline ----------------------------------
            fps = [None, None]
            cps = [None, None]
            mts = [None]

            def issue_fuse(s):
                if s % 8 == 0 and s >= 8 and s + 24 < NT:
                    y_chunk(s // 8 + 3)
                if s % 16 == 0 and s + 16 < NT:
                    x_chunk(s // 16 + 1)
                fps[s % 2] = ps_fuse.tile([128, W], f32, tag="fuse",
                                          name=f"fps{s}")
                lf = LFV[1] if s == 0 else (LFV[2] if s == NT - 1 else LFV[0])
                rhs = yR[:, (s % NYR) * 2 * W:(s % NYR + 1) * 2 * W] \
                    .rearrange("p (two n) -> p two n", two=2)
                if FUSE_PRIO:
                    with tc.high_priority():
                        nc.tensor.matmul(fps[s % 2][:, :], lhsT=lf, rhs=rhs,
                                         start=True, stop=True, perf_mode=DR)
                else:
                    nc.tensor.matmul(fps[s % 2][:, :], lhsT=lf, rhs=rhs,
                                     start=True, stop=True, perf_mode=DR)

            def issue_fuse_evac(s):
                if FACT_MOD and s % FACT_MOD == FACT_MOD - 1:
                    nc.scalar.activation(
                        out=xf[:, (2 * s + 1) * W:(2 * s + 2) * W],
                        in_=fps[s % 2][:, :], func=AF.Relu,
                        accum_out=Ra[:, s:s + 1])
                else:
                    nc.vector.tensor_scalar(
                        out=xf[:, (2 * s + 1) * W:(2 * s + 2) * W],
                        in0=fps[s % 2][:, :], scalar1=0.0, scalar2=0.0,
                        op0=ALU.max, op1=ALU.add, accum_out=Ra[:, s:s + 1])

            def issue_front(f):
                if f % 2 == 0:
                    cps[(f // 2) % CONV_BUFS] = ps_conv.tile(
                        [96, 2 * W], f32, tag="conv", name=f"cps{f//2}")
                t = cps[(f // 2) % CONV_BUFS]
                o = (f % 2) * W
                pv = xf[:, f * 2 * W:(f + 1) * 2 * W] \
                    .rearrange("p (two n) -> p two n", two=2)
                nc.tensor.matmul(t[:, o:o + W], lhsT=WDR[1],
                                 rhs=pv, start=True, stop=False, perf_mode=DR)
                nc.tensor.matmul(t[:, o + 1:o + W], lhsT=WDR[0],
                                 rhs=pv[:, :, 0:W - 1],
                                 start=False, stop=False, perf_mode=DR)
                nc.tensor.matmul(t[:, o:o + W - 1], lhsT=WDR[2],
                                 rhs=pv[:, :, 1:W],
                                 start=False, stop=True, perf_mode=DR)

            def issue_conv_evac(c):
                if c >= NV - DVE_TAIL:
                    nc.vector.tensor_scalar(
                        out=fcc[0:96, 2 * c * W:(2 * c + 2) * W],
                        in0=cps[c % CONV_BUFS][:, :], scalar1=fcb[:, :],
                        scalar2=0.0, op0=ALU.add, op1=ALU.max)
                else:
                    nc.scalar.activation(
                        out=fcc[0:96, 2 * c * W:(2 * c + 2) * W],
                        in_=cps[c % CONV_BUFS][:, :], func=AF.Relu,
                        bias=fcb[:, :])

            def issue_mask(u):
                j = u % MGS
                if j == 0:
                    mts[0] = ps_mask.tile([70, (MGS // 3) * W], f32,
                                          tag="mask", name=f"mt{u//MGS}")
                nc.tensor.matmul(
                    mts[0][32 * (j % 3):32 * (j % 3) + 6,
                           (j // 3) * W:(j // 3 + 1) * W],
                    lhsT=LM6, rhs=fcc[0:97, u * W:(u + 1) * W],
                    start=True, stop=True)
                if j == MGS - 1 or u == NT - 1:
                    issue_logit_out(u // MGS)

            def issue_logit_out(g):
                sg = sgring.tile([70, (MGS // 3) * W], bf16, tag="sg")
                if COPY_SPLIT and g % COPY_MOD == COPY_MOD - 1:
                    nc.vector.tensor_copy(out=sg[:, :], in_=mts[0][:, :])
                else:
                    nc.scalar.activation(out=sg[:, :], in_=mts[0][:, :],
                                         func=AF.Copy)
                eng = nc.sync if OUT_SP else nc.scalar
                eng.dma_start(out=out_ext[g, :, :], in_=sg[:, :])

            def issue_se():
                # pre-sigmoid SE logits; host applies the sigmoid
                gps = ps_conv.tile([96, 2 * W], f32, tag="conv", name="gps")
                nc.tensor.matmul(gps[0:16, 0:NT], lhsT=SEL, rhs=Ra[:, :],
                                 start=True, stop=True)
                gap = singles.tile([16, 1], f32, tag="gap")
                nc.vector.reduce_sum(out=gap[:, :], in_=gps[0:16, 0:NT],
                                     axis=mybir.AxisListType.X)
                hps = ps_conv.tile([96, 2 * W], f32, tag="conv", name="hps")
                nc.tensor.matmul(hps[0:16, 0:1], lhsT=W1L, rhs=gap[:, :],
                                 start=True, stop=True)
                h = singles.tile([16, 1], f32, tag="h")
                nc.scalar.activation(out=h[:, :], in_=hps[0:16, 0:1],
                                     func=AF.Relu)
                sps = ps_fuse.tile([128, W], f32, tag="fuse", name="sps")
                nc.tensor.matmul(sps[:, 0:1], lhsT=W2R, rhs=h[:, :],
                                 start=True, stop=True)
                se_bc = singles.tile([128, 1], f32, tag="sebc")
                nc.vector.tensor_copy(out=se_bc[:, :], in_=sps[:, 0:1])
                nc.sync.dma_start(out=sep_ext[:, :], in_=se_bc[:, :])

            for s in range(NT + FL + 2 * ML + 4):
                if s == 0:
                    issue_fuse(0)
                if s + 1 < NT:
                    issue_fuse(s + 1)
                if s < NT:
                    issue_fuse_evac(s)
                f = s - FL
                if 0 <= f < NT:
                    issue_front(f)
                if 1 <= f < NT and f % 2 == 1:
                    issue_conv_evac(f // 2)
                if f >= 1 and (f - 1) % 2 == 1:
                    v = (f - 1) // 2 - ML
                    if 0 <= v < NV:
                        issue_mask(2 * v)
                        issue_mask(2 * v + 1)
                if s == NT + 1:
                    issue_se()
    nc.compile()
    return nc


# ----------------------------------------------------------------------------
# host packing / unpacking
# ----------------------------------------------------------------------------

def _pack_inputs(x, y):
    """Per-image Toeplitz layouts (fp8): xp [128, NT*W], yp [21, NT*2W]."""
    B = x.shape[0]
    ridx = 6 * np.arange(NT)[:, None] + np.arange(8)[None, :]

    xpad = np.zeros((B, 16, 6 * NT + 8, W), np.float32)
    xpad[:, :, 1:H + 1, :] = x
    xt = xpad[:, :, ridx, :]                       # [B,16,NT,8,W]
    xp = xt.transpose(0, 2, 3, 1, 4).reshape(B, NT, 128, W) \
           .transpose(0, 2, 1, 3).reshape(B, 128, NT * W).astype(F8)

    ypad = np.zeros((B, 5, 6 * NT + 8, W), np.float32)
    ypad[:, :, 1:H + 1, :] = y
    yt = ypad[:, :, ridx, :].transpose(0, 2, 3, 1, 4)   # [B,NT,8,5,W]
    yp = np.zeros((B, 21, NT, 2, W), np.float32)
    yp[:, 0:20, :, 0, :] = yt[:, :, 0:4].reshape(B, NT, 20, W) \
                             .transpose(0, 2, 1, 3)
    yp[:, 0:20, :, 1, :] = yt[:, :, 4:8].reshape(B, NT, 20, W) \
                             .transpose(0, 2, 1, 3)
    yp[:, 20, :, 0, :] = 1.0
    yp = yp.reshape(B, 21, NT * 2 * W).astype(F8)
    return xp, yp


def _decode_out(ot, sep, y, bd_w, bd_b, cv_w, cv_b):
    """Logits [NG, 70, 2W] + se vector + y -> [16, H, W] f32 output."""
    ot = np.asarray(ot, np.float32)
    L = np.zeros((NG * MGS, 6, W), np.float32)
    for j in range(MGS):
        L[j::MGS] = ot[:, 32 * (j % 3):32 * (j % 3) + 6,
                       (j // 3) * W:(j // 3 + 1) * W]
    L = L[:NT]                                      # [NT, 6, W] mask logits
    m = np.clip(L.reshape(NT * SB, W)[:H], -60.0, 60.0)
    sgm = 1.0 / (1.0 + np.exp(-m))

    sl = np.clip(np.asarray(sep, np.float32)[97:102, 0], -60.0, 60.0)
    se = 1.0 / (1.0 + np.exp(-sl))                  # [5]
    db = (bd_w[1, :, 0, 0] - bd_w[0, :, 0, 0]) * se
    bl = np.einsum("c,chw->hw", db, y) + (bd_b[1] - bd_b[0])
    sgb = 1.0 / (1.0 + np.exp(-np.clip(bl, -60.0, 60.0)))

    s = np.minimum(sgm + sgb, 1.0)
    return cv_w[:, 0, 0, 0, None, None] * s[None] + cv_b[:, None, None]


# ----------------------------------------------------------------------------
# entry point
# ----------------------------------------------------------------------------

LAST_RESULT = None


def prepare(x, y, fuse_w, fuse_b, se_w1, se_w2, bd_w, bd_b,
            fc_w, fc_b, fm_w, fm_b, cv_w, cv_b):
    if "nc" not in _cache:
        _cache["nc"] = _build()
    nc = _cache["nc"]

    g = lambda v: np.asarray(v, np.float32)
    w8 = _pack_w8(g(fuse_w), g(fuse_b), g(fc_w), g(fm_w), g(fm_b))
    w32 = _se_consts(g(se_w1), g(se_w2))
    fcb = _fcb_col(g(fc_b))
    ct8 = np.ones((1, NT * W), np.float32).astype(F8)

    xp, yp = _pack_inputs(g(x), g(y))
    in_maps = [
        {"xp": np.ascontiguousarray(xp[i]),
         "yp": np.ascontiguousarray(yp[i]),
         "w8": w8, "w32": w32, "fcb": fcb, "ct8": ct8}
        for i in range(x.shape[0])
    ]
    return nc, in_maps


def kernel(x, y, fuse_w, fuse_b, se_w1, se_w2, bd_w, bd_b,
           fc_w, fc_b, fm_w, fm_b, cv_w, cv_b):
    global LAST_RESULT
    from concourse.bass_utils import run_bass_kernel_spmd

    nc, in_maps = prepare(x, y, fuse_w, fuse_b, se_w1, se_w2, bd_w, bd_b,
                          fc_w, fc_b, fm_w, fm_b, cv_w, cv_b)
    res = run_bass_kernel_spmd(nc, in_maps, core_ids=list(range(8)))
    LAST_RESULT = res
    gw = np.asarray(bd_w, np.float32)
    gb = np.asarray(bd_b, np.float32)
    cw = np.asarray(cv_w, np.float32)
    cb = np.asarray(cv_b, np.float32)
    yf = np.asarray(y, np.float32)
    outs = [_decode_out(res.results[i]["outp"], res.results[i]["sep"],
                        yf[i], gw, gb, cw, cb)
            for i in range(len(in_maps))]
    return np.stack(outs).astype(np.float32)


# revision 65
# speedup vs baseline: 1.0093x; 1.0093x over previous
"""Trainium2 Bass kernel for nn_Boundary_Enchance (dense_cnn).

Pure data parallel: core i of 8 processes batch image i.  The heavy matmul
work runs in fp8(e4m3) DoubleRow mode (2 weight planes per PE cell, K_eff=256,
0.5 cycles/col), 4x the bf16 column rate:

  - fuse 1x1 conv (5->16 + bias + relu): one DoubleRow matmul per 8-row
    strip; pair halves = y rows 0..3 / rows 4..7 (+ ones partition for the
    bias).  Evacuated (relu, fp8) by the Vector engine with accum_out row
    sums feeding the global-average-pool / SE path.
  - 3x3 conv over concat(x, fuse): 3 DoubleRow matmuls per strip (one per
    dx tap); pair halves = (x rows, fuse rows) interleaved per strip in one
    big SBUF region so the pair stride is a constant 512.  dy taps are
    packed row-Toeplitz in the 128 partitions; SAME padding via
    partial-column accumulating matmuls.  Evacuated (bias+relu, fp8) by the
    Scalar engine, 2 strips per op, into the fcc region.
  - mask head: one plain fp8 matmul per 2 strips (K=97: 96 fcc rows +
    ones; M=6 mask-diff logits; N=1024) — SE-independent, so it interleaves
    with the conv pipeline.  Three matmuls share a 2-bank PSUM tile at
    partition bases 0/32/64; one Copy op + one DMA evacuates 6 strips of
    logits as bf16.
  - SE chain stays on device (row sums -> selection matmul -> MLP ->
    sigmoid); the 128-wide sigmoid vector is DMA'd back.

The host does layout packing (fp8 Toeplitz tiles) and the cheap epilogue:
sigmoid on the mask logits, the 5-channel boundary head (1x1 conv with
device-provided SE scale + sigmoid), add, clip, and the final rank-1 1x1
16-channel expansion (out = cv_w * s + cv_b).
"""

import numpy as np
import ml_dtypes

F8 = ml_dtypes.float8_e4m3
BF16 = ml_dtypes.bfloat16

H = 512
W = 512
SB = 6                     # output rows per strip
NT = (H + SB - 1) // SB    # 86 strips
NV = NT // 2               # 43 mask matmuls (2 strips each)
NPIX = float(H * W)
MGS = 6                    # strips per logit tile (3 or 6)
CONV_BUFS = 2
COPY_PRIO = 0              # priority offset for logit copies (0 = none)
TBL_PRELOAD = True         # dummy activation to preload the act table early
FUSE_PRIO = True           # schedule fuse matmuls ahead of all other PE work
MASK_BUFS = 1
COPY_SPLIT = True
COPY_MOD = 4
OUT_SP = True              # out DMAs on the SP queue instead of Act
DVE_TAIL = 4               # last conv-evac pairs handled by DVE
SGB = 5                    # logit staging ring depth
FACT_MOD = 0               # every Nth fuse evac on Act (0 = all on DVE)
NG = (NT + MGS - 1) // MGS
NYR = 32                   # y ring depth (strips)
FL = 3                     # fuse -> conv front lag (strips)
ML = 2                     # conv-evac pair -> mask matmul lag (pairs)

_cache = {}


# ----------------------------------------------------------------------------
# host-side weight layout builders
# ----------------------------------------------------------------------------

def _conv_pair_lhsT(fc_w):
    """[3][128, 192]: cols 0-95 x-half, 96-191 F-half.
    W[dx][r*16+c, half*96 + i*16+oc] = fc_w[oc, half*16+c, r-i, dx]."""
    out = np.zeros((3, 128, 192), np.float32)
    for dx in range(3):
        for half in range(2):
            for i in range(SB):
                for ky in range(3):
                    r = i + ky
                    out[dx, r * 16:r * 16 + 16,
                        half * 96 + i * 16:half * 96 + i * 16 + 16] = \
                        fc_w[:, half * 16:half * 16 + 16, ky, dx].T
    return out


def _fuse_pair_lhsT(fuse_w, fuse_b, zero_out_rows=()):
    """[21, 256]: half0 (cols 0-127) = y rows 0-3 + bias on partition 20;
    half1 (cols 128-255) = y rows 4-7.  lhsT[r%4*5+yc, half*128 + r*16+oc].
    zero_out_rows: tile rows r whose output (and bias) must be zeroed."""
    out = np.zeros((21, 256), np.float32)
    for r in range(8):
        if r in zero_out_rows:
            continue
        half = r // 4
        q = (r % 4) * 5
        out[q:q + 5, half * 128 + r * 16:half * 128 + r * 16 + 16] = \
            fuse_w[:, :, 0, 0].T
        out[20, r * 16:r * 16 + 16] = fuse_b  # bias lives in half0
    return out


def _lm6(fm_w, fm_b):
    """Static mask head [128, 6]: cols = the 6 output rows of a strip."""
    lm = np.zeros((128, 6), np.float32)
    dm = fm_w[1, :, 0, 0] - fm_w[0, :, 0, 0]
    for i in range(SB):
        lm[i * 16:i * 16 + 16, i] = dm
    lm[96, 0:6] = fm_b[1] - fm_b[0]
    return lm


def _se_consts(se_w1, se_w2):
    """[128, 160] f32: SEL (cols 0-15), W1L (16-31), W2R (32-159)."""
    out = np.zeros((128, 160), np.float32)
    for r in range(1, 7):
        for fc in range(16):
            out[r * 16 + fc, fc] = 1.0 / NPIX
    out[0:16, 16:32] = se_w1.T
    out[0:16, 32 + 97:32 + 102] = se_w2.T
    return out


def _fcb_col(fc_b):
    out = np.zeros((96, 1), np.float32)
    for i in range(SB):
        out[i * 16:(i + 1) * 16, 0] = fc_b
    return out


def _pack_w8(fuse_w, fuse_b, fc_w, fm_w, fm_b):
    """[128, 1360] fp8: 3 conv pair blocks (192 each), 3 fuse variants
    (256 each): LF, LF_first (row -1 zeroed), LF_last (rows 3.. zeroed),
    then the static mask head LM6 (cols 1344-1349)."""
    out = np.zeros((128, 1360), np.float32)
    cw = _conv_pair_lhsT(fc_w)
    for dx in range(3):
        out[:, dx * 192:(dx + 1) * 192] = cw[dx]
    base = 3 * 192
    out[0:21, base:base + 256] = _fuse_pair_lhsT(fuse_w, fuse_b)
    out[0:21, base + 256:base + 512] = _fuse_pair_lhsT(
        fuse_w, fuse_b, zero_out_rows=(0,))
    out[0:21, base + 512:base + 768] = _fuse_pair_lhsT(
        fuse_w, fuse_b, zero_out_rows=(3, 4, 5, 6, 7))
    out[:, 1344:1350] = _lm6(fm_w, fm_b)
    return out.astype(F8)


# ----------------------------------------------------------------------------
# bass graph
# ----------------------------------------------------------------------------

def _build():
    import concourse.bass as bass
    import concourse.bacc as bacc
    import concourse.tile as tile
    from concourse import mybir

    f32 = mybir.dt.float32
    bf16 = mybir.dt.bfloat16
    fp8 = mybir.dt.float8e4
    AF = mybir.ActivationFunctionType
    ALU = mybir.AluOpType
    DR = mybir.MatmulPerfMode.DoubleRow

    nc = bacc.Bacc("TRN2", target_bir_lowering=False)
    xp_ext = nc.declare_dram_parameter("xp", [128, NT * W], fp8, isOutput=False)
    yp_ext = nc.declare_dram_parameter("yp", [21, NT * 2 * W], fp8,
                                       isOutput=False)
    w8_ext = nc.declare_dram_parameter("w8", [128, 1360], fp8, isOutput=False)
    w32_ext = nc.declare_dram_parameter("w32", [128, 160], f32, isOutput=False)
    fcb_ext = nc.declare_dram_parameter("fcb", [96, 1], f32, isOutput=False)
    ct8_ext = nc.declare_dram_parameter("ct8", [1, NT * W], fp8, isOutput=False)
    out_ext = nc.declare_dram_parameter("outp", [NG, 70, (MGS // 3) * W],
                                        bf16, isOutput=True)
    sep_ext = nc.declare_dram_parameter("sep", [128, 1], f32, isOutput=True)

    with tile.TileContext(nc) as tc:
        with (
            tc.tile_pool(name="singles", bufs=1) as singles,
            tc.tile_pool(name="sgring", bufs=SGB) as sgring,
            tc.tile_pool(name="ps_fuse", bufs=2, space="PSUM") as ps_fuse,
            tc.tile_pool(name="ps_conv", bufs=CONV_BUFS,
                         space="PSUM") as ps_conv,
            tc.tile_pool(name="ps_mask", bufs=MASK_BUFS,
                         space="PSUM") as ps_mask,
        ):
            # ---------------- constants + first data chunks -------------
            w8 = singles.tile([128, 1360], fp8, tag="w8")
            nc.sync.dma_start(out=w8[:, :], in_=w8_ext[:, :])
            if TBL_PRELOAD:
                # touch Relu early so the act-table load is off the
                # critical path of the first conv evacuation
                tpre = singles.tile([1, 1], f32, tag="tpre")
                nc.scalar.activation(out=tpre[:, :], in_=tpre[:, :],
                                     func=AF.Relu)
            yR = singles.tile([21, NYR * 2 * W], fp8, tag="yR")
            xf = singles.tile([128, NT * 2 * W], fp8, tag="xf")
            fcc = singles.tile([128, NT * W], fp8, tag="fcc")

            def x_range(s0, n):
                v = xf[:, 2 * s0 * W:2 * (s0 + n) * W] \
                    .rearrange("p (s j) -> p s j", j=2 * W)[:, :, 0:W]
                nc.sync.dma_start(
                    out=v,
                    in_=xp_ext[:, s0 * W:(s0 + n) * W]
                    .rearrange("p (s j) -> p s j", j=W))

            def x_chunk(k):
                x_range(16 * k, min(16, NT - 16 * k))

            def y_chunk(k):
                n = min(8, NT - 8 * k)
                r0 = (8 * k) % NYR
                nc.sync.dma_start(
                    out=yR[:, r0 * 2 * W:(r0 + n) * 2 * W],
                    in_=yp_ext[:, 8 * k * 2 * W:(8 * k + n) * 2 * W])

            y_chunk(0)
            y_chunk(1)
            x_range(0, 8)
            fcb = singles.tile([96, 1], f32, tag="fcb")
            nc.sync.dma_start(out=fcb[:, :], in_=fcb_ext[:, :])
            w32 = singles.tile([128, 160], f32, tag="w32")
            nc.sync.dma_start(out=w32[:, :], in_=w32_ext[:, :])
            y_chunk(2)
            x_range(8, 8)
            nc.sync.dma_start(out=fcc[96:97, :], in_=ct8_ext[:, :])
            y_chunk(3)

            WDR = [w8[:, dx * 192:(dx + 1) * 192]
                   .rearrange("p (two m) -> p two m", two=2) for dx in range(3)]
            fb = 3 * 192
            LFV = [w8[0:21, fb + v * 256:fb + (v + 1) * 256]
                   .rearrange("p (two m) -> p two m", two=2) for v in range(3)]
            LM6 = w8[0:97, 1344:1350]
            SEL = w32[:, 0:16]
            W1L = w32[0:16, 16:32]
            W2R = w32[0:16, 32:160]

            Ra = singles.tile([128, NT], f32, tag="Ra")

            # ---------------- pipeline ----------------------------------
            fps = [None, None]
            cps = [None, None]
            mts = [None]

            def issue_fuse(s):
                if s % 8 == 0 and s >= 8 and s + 24 < NT:
                    y_chunk(s // 8 + 3)
                if s % 16 == 0 and s + 16 < NT:
                    x_chunk(s // 16 + 1)
                fps[s % 2] = ps_fuse.tile([128, W], f32, tag="fuse",
                                          name=f"fps{s}")
                lf = LFV[1] if s == 0 else (LFV[2] if s == NT - 1 else LFV[0])
                rhs = yR[:, (s % NYR) * 2 * W:(s % NYR + 1) * 2 * W] \
                    .rearrange("p (two n) -> p two n", two=2)
                if FUSE_PRIO:
                    with tc.high_priority():
                        nc.tensor.matmul(fps[s % 2][:, :], lhsT=lf, rhs=rhs,
                                         start=True, stop=True, perf_mode=DR)
                else:
                    nc.tensor.matmul(fps[s % 2][:, :], lhsT=lf, rhs=rhs,
                                     start=True, stop=True, perf_mode=DR)

            def issue_fuse_evac(s):
                if FACT_MOD and s % FACT_MOD == FACT_MOD - 1:
                    nc.scalar.activation(
                        out=xf[:, (2 * s + 1) * W:(2 * s + 2) * W],
                        in_=fps[s % 2][:, :], func=AF.Relu,
                        accum_out=Ra[:, s:s + 1])
                else:
                    nc.vector.tensor_scalar(
                        out=xf[:, (2 * s + 1) * W:(2 * s + 2) * W],
                        in0=fps[s % 2][:, :], scalar1=0.0, scalar2=0.0,
                        op0=ALU.max, op1=ALU.add, accum_out=Ra[:, s:s + 1])

            def issue_front(f):
                if f % 2 == 0:
                    cps[(f // 2) % CONV_BUFS] = ps_conv.tile(
                        [96, 2 * W], f32, tag="conv", name=f"cps{f//2}")
                t = cps[(f // 2) % CONV_BUFS]
                o = (f % 2) * W
                pv = xf[:, f * 2 * W:(f + 1) * 2 * W] \
                    .rearrange("p (two n) -> p two n", two=2)
                nc.tensor.matmul(t[:, o:o + W], lhsT=WDR[1],
                                 rhs=pv, start=True, stop=False, perf_mode=DR)
                nc.tensor.matmul(t[:, o + 1:o + W], lhsT=WDR[0],
                                 rhs=pv[:, :, 0:W - 1],
                                 start=False, stop=False, perf_mode=DR)
                nc.tensor.matmul(t[:, o:o + W - 1], lhsT=WDR[2],
                                 rhs=pv[:, :, 1:W],
                                 start=False, stop=True, perf_mode=DR)

            def issue_conv_evac(c):
                if c >= NV - DVE_TAIL:
                    nc.vector.tensor_scalar(
                        out=fcc[0:96, 2 * c * W:(2 * c + 2) * W],
                        in0=cps[c % CONV_BUFS][:, :], scalar1=fcb[:, :],
                        scalar2=0.0, op0=ALU.add, op1=ALU.max)
                else:
                    nc.scalar.activation(
                        out=fcc[0:96, 2 * c * W:(2 * c + 2) * W],
                        in_=cps[c % CONV_BUFS][:, :], func=AF.Relu,
                        bias=fcb[:, :])

            def issue_mask(u):
                j = u % MGS
                if j == 0:
                    mts[0] = ps_mask.tile([70, (MGS // 3) * W], f32,
                                          tag="mask", name=f"mt{u//MGS}")
                nc.tensor.matmul(
                    mts[0][32 * (j % 3):32 * (j % 3) + 6,
                           (j // 3) * W:(j // 3 + 1) * W],
                    lhsT=LM6, rhs=fcc[0:97, u * W:(u + 1) * W],
                    start=True, stop=True)
                if j == MGS - 1 or u == NT - 1:
                    issue_logit_out(u // MGS)

            def issue_logit_out(g):
                sg = sgring.tile([70, (MGS // 3) * W], bf16, tag="sg")
                nr = 70 if g < NG - 1 else 32 * (((NT - 1) % MGS) % 3) + 6
                ncol = (MGS // 3) * W if g < NG - 1 else                     (((NT - 1) % MGS) // 3 + 1) * W
                if COPY_SPLIT and g % COPY_MOD == COPY_MOD - 1:
                    nc.vector.tensor_copy(out=sg[0:nr, 0:ncol],
                                          in_=mts[0][0:nr, 0:ncol])
                else:
                    nc.scalar.activation(out=sg[0:nr, 0:ncol],
                                         in_=mts[0][0:nr, 0:ncol],
                                         func=AF.Copy)
                eng = nc.sync if OUT_SP else nc.scalar
                eng.dma_start(out=out_ext[g, 0:nr, 0:ncol],
                              in_=sg[0:nr, 0:ncol])

            def issue_se():
                # pre-sigmoid SE logits; host applies the sigmoid
                gps = ps_conv.tile([96, 2 * W], f32, tag="conv", name="gps")
                nc.tensor.matmul(gps[0:16, 0:NT], lhsT=SEL, rhs=Ra[:, :],
                                 start=True, stop=True)
                gap = singles.tile([16, 1], f32, tag="gap")
                nc.vector.reduce_sum(out=gap[:, :], in_=gps[0:16, 0:NT],
                                     axis=mybir.AxisListType.X)
                hps = ps_conv.tile([96, 2 * W], f32, tag="conv", name="hps")
                nc.tensor.matmul(hps[0:16, 0:1], lhsT=W1L, rhs=gap[:, :],
                                 start=True, stop=True)
                h = singles.tile([16, 1], f32, tag="h")
                nc.scalar.activation(out=h[:, :], in_=hps[0:16, 0:1],
                                     func=AF.Relu)
                sps = ps_fuse.tile([128, W], f32, tag="fuse", name="sps")
                nc.tensor.matmul(sps[:, 0:1], lhsT=W2R, rhs=h[:, :],
                                 start=True, stop=True)
                se_bc = singles.tile([128, 1], f32, tag="sebc")
                nc.vector.tensor_copy(out=se_bc[:, :], in_=sps[:, 0:1])
                nc.sync.dma_start(out=sep_ext[:, :], in_=se_bc[:, :])

            for s in range(NT + FL + 2 * ML + 4):
                if s == 0:
                    issue_fuse(0)
                if s + 1 < NT:
                    issue_fuse(s + 1)
                if s < NT:
                    issue_fuse_evac(s)
                f = s - FL
                if 0 <= f < NT:
                    issue_front(f)
                if 1 <= f < NT and f % 2 == 1:
                    issue_conv_evac(f // 2)
                if f >= 1 and (f - 1) % 2 == 1:
                    v = (f - 1) // 2 - ML
                    if 0 <= v < NV:
                        issue_mask(2 * v)
                        issue_mask(2 * v + 1)
                if s == NT + 1:
                    issue_se()
    nc.compile()
    return nc


# ----------------------------------------------------------------------------
# host packing / unpacking
# ----------------------------------------------------------------------------

def _pack_inputs(x, y):
    """Per-image Toeplitz layouts (fp8): xp [128, NT*W], yp [21, NT*2W]."""
    B = x.shape[0]
    ridx = 6 * np.arange(NT)[:, None] + np.arange(8)[None, :]

    xpad = np.zeros((B, 16, 6 * NT + 8, W), np.float32)
    xpad[:, :, 1:H + 1, :] = x
    xt = xpad[:, :, ridx, :]                       # [B,16,NT,8,W]
    xp = xt.transpose(0, 2, 3, 1, 4).reshape(B, NT, 128, W) \
           .transpose(0, 2, 1, 3).reshape(B, 128, NT * W).astype(F8)

    ypad = np.zeros((B, 5, 6 * NT + 8, W), np.float32)
    ypad[:, :, 1:H + 1, :] = y
    yt = ypad[:, :, ridx, :].transpose(0, 2, 3, 1, 4)   # [B,NT,8,5,W]
    yp = np.zeros((B, 21, NT, 2, W), np.float32)
    yp[:, 0:20, :, 0, :] = yt[:, :, 0:4].reshape(B, NT, 20, W) \
                             .transpose(0, 2, 1, 3)
    yp[:, 0:20, :, 1, :] = yt[:, :, 4:8].reshape(B, NT, 20, W) \
                             .transpose(0, 2, 1, 3)
    yp[:, 20, :, 0, :] = 1.0
    yp = yp.reshape(B, 21, NT * 2 * W).astype(F8)
    return xp, yp


def _decode_out(ot, sep, y, bd_w, bd_b, cv_w, cv_b):
    """Logits [NG, 70, 2W] + se vector + y -> [16, H, W] f32 output."""
    ot = np.asarray(ot, np.float32)
    L = np.zeros((NG * MGS, 6, W), np.float32)
    for j in range(MGS):
        L[j::MGS] = ot[:, 32 * (j % 3):32 * (j % 3) + 6,
                       (j // 3) * W:(j // 3 + 1) * W]
    L = L[:NT]                                      # [NT, 6, W] mask logits
    m = np.clip(L.reshape(NT * SB, W)[:H], -60.0, 60.0)
    sgm = 1.0 / (1.0 + np.exp(-m))

    sl = np.clip(np.asarray(sep, np.float32)[97:102, 0], -60.0, 60.0)
    se = 1.0 / (1.0 + np.exp(-sl))                  # [5]
    db = (bd_w[1, :, 0, 0] - bd_w[0, :, 0, 0]) * se
    bl = np.einsum("c,chw->hw", db, y) + (bd_b[1] - bd_b[0])
    sgb = 1.0 / (1.0 + np.exp(-np.clip(bl, -60.0, 60.0)))

    s = np.minimum(sgm + sgb, 1.0)
    return cv_w[:, 0, 0, 0, None, None] * s[None] + cv_b[:, None, None]


# ----------------------------------------------------------------------------
# entry point
# ----------------------------------------------------------------------------

LAST_RESULT = None


def prepare(x, y, fuse_w, fuse_b, se_w1, se_w2, bd_w, bd_b,
            fc_w, fc_b, fm_w, fm_b, cv_w, cv_b):
    if "nc" not in _cache:
        _cache["nc"] = _build()
    nc = _cache["nc"]

    g = lambda v: np.asarray(v, np.float32)
    w8 = _pack_w8(g(fuse_w), g(fuse_b), g(fc_w), g(fm_w), g(fm_b))
    w32 = _se_consts(g(se_w1), g(se_w2))
    fcb = _fcb_col(g(fc_b))
    ct8 = np.ones((1, NT * W), np.float32).astype(F8)

    xp, yp = _pack_inputs(g(x), g(y))
    in_maps = [
        {"xp": np.ascontiguousarray(xp[i]),
         "yp": np.ascontiguousarray(yp[i]),
         "w8": w8, "w32": w32, "fcb": fcb, "ct8": ct8}
        for i in range(x.shape[0])
    ]
    return nc, in_maps


def kernel(x, y, fuse_w, fuse_b, se_w1, se_w2, bd_w, bd_b,
           fc_w, fc_b, fm_w, fm_b, cv_w, cv_b):
    global LAST_RESULT
    from concourse.bass_utils import run_bass_kernel_spmd

    nc, in_maps = prepare(x, y, fuse_w, fuse_b, se_w1, se_w2, bd_w, bd_b,
                          fc_w, fc_b, fm_w, fm_b, cv_w, cv_b)
    res = run_bass_kernel_spmd(nc, in_maps, core_ids=list(range(8)))
    LAST_RESULT = res
    gw = np.asarray(bd_w, np.float32)
    gb = np.asarray(bd_b, np.float32)
    cw = np.asarray(cv_w, np.float32)
    cb = np.asarray(cv_b, np.float32)
    yf = np.asarray(y, np.float32)
    outs = [_decode_out(res.results[i]["outp"], res.results[i]["sep"],
                        yf[i], gw, gb, cw, cb)
            for i in range(len(in_maps))]
    return np.stack(outs).astype(np.float32)
